# revision 7
# baseline (speedup 1.0000x reference)
"""DSMIL pooling kernel for 8 Trainium2 NeuronCores — folded-algebra fp8 design.

Algebraic folding (exact): with h = x@we + be,
  inst_logits = h@wi + bi           = x @ (we@wi) + (be@wi + bi)
  scores      = q·(h@wk + bk)/√E    = x @ (we@(wk·q))/√E + const  (const cancels in softmax)
  attn_bag    = Σ attn_n h_n        = (Σ attn_n x_n) @ we + be    (Σ attn_n = 1)
So the device only needs three thin, memory-bound x-contractions:
  L1: l = x @ wei      (per-instance logits; host does argmax + exact f32
      recheck of the top-32 noisy candidates, then critical/q/v folds)
  L2: s = x @ wev  ->  w = exp(s)  ->  wX = Σ w_n x_n   (one launch)
x is streamed in fp8e4m3 (x·16), weights fp8 (wei·512, wev·1024); the host
applies exact power-of-2 unscaling. DoubleRow matmuls contract 256 rows per
instruction (dual-fp8 ldweights need stationary free ≥16 and small dual-dim
strides, hence the padded wei and the n = blk*256 + i*128 + p blocking).
Host glue is O(E²) weight folds plus one O(32·D) recheck.

Sharding: core c <- (bag c//2, half c%2), each shard NS=8192 instances.
"""

import numpy as np
import ml_dtypes

import concourse.mybir as mybir
import concourse.tile as tile
from concourse import bacc
from concourse.bass_utils import run_bass_kernel_spmd

# ---- tile-tail drain workaround (this walrus build rejects >1 sync-wait
# per instruction on the kernel-tail Drain) ----
from concourse.vector_clock import ScopedClock

_MAX_WAITS = 1


def _patched_drain_and_barrier(self, tick_clock, wait_clock):
    probe = self.nc.sync.nop(nofuse=True, hint="tile_drain_waits")
    wait_clock.add_sem_waits(probe.ins, ScopedClock({None: tick_clock.global_clock}))
    si = probe.ins.sync_info
    waits = list(si.on_wait) if si is not None and si.on_wait else []
    if len(waits) > _MAX_WAITS:
        si.on_wait = waits[:_MAX_WAITS]
        rest = waits[_MAX_WAITS:]
        for k in range(0, len(rest), _MAX_WAITS):
            extra = self.nc.sync.nop(nofuse=True, hint="tile_drain_waits")
            esi = extra.ins.sync_info
            if esi is None:
                extra.ins.sync_info = mybir.SyncInfo(
                    on_wait=rest[k : k + _MAX_WAITS], on_update=[]
                )
            else:
                esi.on_wait = rest[k : k + _MAX_WAITS]
    self.nc.sync.drain()
    self.nc.all_engine_barrier()
    popped = self.nc._tile_sem_poison_stack.pop()
    assert popped is self._sem_poison
    self.nc.clear_and_free_semaphores(list(self.sems.allocated().values()))
    self.nc.all_engine_barrier()


tile.TileContext._drain_and_barrier = _patched_drain_and_barrier

F32 = mybir.dt.float32
F8 = mybir.dt.float8e4
FP8 = ml_dtypes.float8_e4m3

B, N, D, E, C = 4, 16384, 1024, 512, 2
NCORES = 8
NS = N // 2          # per-core sequence shard
NT = 512             # n-tile for the xT stream
NTILES = NS // NT    # 16
PR = 4               # 256-row DoubleRow blocks along D (d = pr*256 + i*128 + p)
NBLK = NS // 256     # 32  (n = blk*256 + i*128 + p)
DC = D // 128        # 8
CPAD = 16            # wei columns padded up to the dual-fp8 ldweights minimum

XS = 16.0            # x fp8 scale
WEIS = 512.0         # wei fp8 scale
WEVS = 1024.0        # wev fp8 scale

_cache = {}


def _build_l1():
    nc = bacc.Bacc(None, target_bir_lowering=False)
    xt_d = nc.dram_tensor("xt", [128, NTILES, PR, 2, NT], F8, kind="ExternalInput")
    wei_d = nc.dram_tensor("wei", [128, PR, 2, CPAD], F8, kind="ExternalInput")
    l_d = nc.dram_tensor("l", [C, NTILES, NT], F32, kind="ExternalOutput")

    with tile.TileContext(nc) as tc:
        with (
            tc.tile_pool(name="wp", bufs=1) as wp,
            tc.tile_pool(name="xp", bufs=3) as xp,
            tc.tile_pool(name="ps", bufs=4, space="PSUM") as pp,
        ):
            wei_sb = wp.tile([128, PR, 2, CPAD], F8)
            nc.sync.dma_start(wei_sb[:], wei_d[:])
            lstack = wp.tile([C, NTILES, NT], F32)

            for k in range(NTILES // 2):
                x_t = xp.tile([128, 2, PR, 2, NT], F8, tag="xt", name="xt")
                nc.sync.dma_start(x_t[:], xt_d[:, 2 * k : 2 * k + 2])
                for t2 in range(2):
                    nt = 2 * k + t2
                    ps = pp.tile([CPAD, NT], F32, tag="l")
                    for pr in range(PR):
                        nc.tensor.matmul(
                            ps[:],
                            lhsT=wei_sb[:, pr],
                            rhs=x_t[:, t2, pr],
                            start=(pr == 0),
                            stop=(pr == PR - 1),
                            perf_mode=mybir.MatmulPerfMode.DoubleRow,
                        )
                    nc.scalar.copy(lstack[:, nt, :], ps[0:C, :])
            nc.sync.dma_start(l_d[:], lstack[:])
    nc.compile()
    return nc


def _build_l2():
    nc = bacc.Bacc(None, target_bir_lowering=False)
    xt_d = nc.dram_tensor("xt", [128, NTILES, PR, 2, NT], F8, kind="ExternalInput")
    xn_d = nc.dram_tensor("xn", [128, NBLK, 2, D], F8, kind="ExternalInput")
    wev_d = nc.dram_tensor("wev", [128, PR, 2, 1], F8, kind="ExternalInput")
    u_d = nc.dram_tensor("u", [1, D], F32, kind="ExternalOutput")
    ssum_d = nc.dram_tensor("ssum", [128, 1], F32, kind="ExternalOutput")

    with tile.TileContext(nc) as tc:
        with (
            tc.tile_pool(name="wp", bufs=1) as wp,
            tc.tile_pool(name="xp", bufs=3) as xp,
            tc.tile_pool(name="pw", bufs=1, space="PSUM") as pwp,
        ):
            wev_sb = wp.tile([128, PR, 2, 1], F8)
            nc.sync.dma_start(wev_sb[:], wev_d[:])

            # ---- phase A: s = x @ wev, directly in n-partition layout.
            # lhsT = x-chunk (stationary), rhs = wev (moving):
            # wps[p, i, blk] = s[blk*256 + i*128 + p] (raw, scaled by XS*WEVS).
            wps = pwp.tile([128, 2, NBLK, 1], F32, tag="w")
            for k in range(NTILES // 2):
                x_t = xp.tile([128, 2, PR, 2, NT], F8, tag="xt", name="xt")
                nc.sync.dma_start(x_t[:], xt_d[:, 2 * k : 2 * k + 2])
                for t2 in range(2):
                    nt = 2 * k + t2
                    for jc in range(4):
                        for pr in range(PR):
                            nc.tensor.matmul(
                                wps[:, jc % 2, 2 * nt + jc // 2],
                                lhsT=x_t[:, t2, pr, :, jc * 128 : (jc + 1) * 128],
                                rhs=wev_sb[:, pr],
                                start=(pr == 0),
                                stop=(pr == PR - 1),
                                perf_mode=mybir.MatmulPerfMode.DoubleRow,
                            )

            # xn streamed in chunks behind the xt tiles (same DMA ring)
            xn_sb = wp.tile([128, NBLK, 2, D], F8)
            for ch in range(4):
                nc.sync.dma_start(
                    xn_sb[:, ch * (NBLK // 4) : (ch + 1) * (NBLK // 4)],
                    xn_d[:, ch * (NBLK // 4) : (ch + 1) * (NBLK // 4)],
                )

            # ---- w = exp(s/16384) quantized to fp8, replicated 16 wide so it
            # can serve as the DoubleRow stationary (ldweights needs free>=16).
            # w_rep[p, i, blk, k] = w[blk*256 + i*128 + p] for all k.
            w_rep = wp.tile([128, 2, NBLK, 16], F8)
            ssum = wp.tile([128, 1], F32)
            for rep in range(16):
                nc.scalar.activation(
                    w_rep[:, :, :, rep : rep + 1], wps[:],
                    mybir.ActivationFunctionType.Exp,
                    scale=1.0 / (XS * WEVS),
                    accum_out=ssum[:] if rep == 0 else None,
                )
            nc.sync.dma_start(ssum_d[:], ssum[:])

            # ---- phase B: wX = Σ w_n x_n.  Stationary = w_rep (16 identical
            # columns), moving = 512-wide xn slices; the two d-half groups
            # accumulate in separate psum banks, so interleaving them while
            # chasing the xn chunk DMAs is safe.
            pu = [
                pwp.tile([16, NT], F32, tag=f"u{dh}", name=f"u{dh}")
                for dh in range(2)
            ]
            for blk in range(NBLK):
                for dh in range(2):
                    nc.tensor.matmul(
                        pu[dh][:],
                        lhsT=w_rep[:, :, blk],
                        rhs=xn_sb[:, blk, :, dh * NT : (dh + 1) * NT],
                        start=(blk == 0),
                        stop=(blk == NBLK - 1),
                        perf_mode=mybir.MatmulPerfMode.DoubleRow,
                    )
            u_sb = wp.tile([1, D], F32)
            for dh in range(2):
                nc.scalar.copy(u_sb[:, dh * NT : (dh + 1) * NT], pu[dh][0:1, :])
            nc.sync.dma_start(u_d[:], u_sb[:])
    nc.compile()
    return nc


def _q8(a, scale, lim=200.0):
    return np.clip(np.asarray(a, np.float32) * scale, -lim, lim).astype(FP8)


def _prep_x(xs):
    """xs [NS, D] f32 -> (xt8 [128, NTILES, PR, 2, NT], xn8 [128, NBLK, 2, D])."""
    xq = _q8(xs, XS)
    xt8 = np.ascontiguousarray(
        xq.reshape(NTILES, NT, PR, 2, 128).transpose(4, 0, 2, 3, 1)
    )
    xn8 = np.ascontiguousarray(
        xq.reshape(NBLK, 2, 128, D).transpose(2, 0, 1, 3)
    )
    return xt8, xn8


def _blk_d(v, scale):
    """[D, m] f32 -> [128, PR, 2, m] fp8 with d = pr*256 + i*128 + p."""
    v = np.asarray(v, np.float32)
    if v.ndim == 1:
        v = v[:, None]
    m = v.shape[1]
    return np.ascontiguousarray(
        _q8(v, scale).reshape(PR, 2, 128, m).transpose(2, 0, 1, 3)
    )


def kernel(x, we, be, wi, bi, wq, bq, wk, bk, wb, bb):
    x = np.asarray(x, dtype=np.float32)
    we = np.asarray(we, dtype=np.float32)
    be = np.asarray(be, dtype=np.float32)
    wi = np.asarray(wi, dtype=np.float32)
    bi = np.asarray(bi, dtype=np.float32)
    wq = np.asarray(wq, dtype=np.float32)
    bq = np.asarray(bq, dtype=np.float32)
    wk = np.asarray(wk, dtype=np.float32)
    bk = np.asarray(bk, dtype=np.float32)
    wb = np.asarray(wb, dtype=np.float32)
    bb = np.asarray(bb, dtype=np.float32)

    if "l1" not in _cache:
        _cache["l1"] = _build_l1()
    if "l2" not in _cache:
        _cache["l2"] = _build_l2()

    wei = we @ wi                       # [D, C]
    bei = be @ wi + bi                  # [C]
    wei_pad = np.zeros((D, CPAD), np.float32)
    wei_pad[:, :C] = wei
    wei8 = _blk_d(wei_pad, WEIS)

    shards = []                         # per-core (xt8, xn8)
    for c in range(NCORES):
        b, h = divmod(c, 2)
        shards.append(_prep_x(x[b, h * NS : (h + 1) * NS]))

    in1 = [{"xt": s[0], "wei": wei8} for s in shards]
    res1 = run_bass_kernel_spmd(_cache["l1"], in1, core_ids=list(range(NCORES))).results

    # ---- host glue: noisy argmax + exact f32 recheck -> critical -> v ----
    scale = np.float32(E) ** 0.5
    crit = [None] * B
    wev8s = [None] * NCORES
    for b in range(B):
        sc_parts = []
        for h in range(2):
            lraw = res1[2 * b + h]["l"]              # [C, NTILES, NT]
            l = lraw.transpose(1, 2, 0).reshape(NS, C) / (XS * WEIS) + bei
            sc_parts.append(l.max(axis=1))
        sc = np.concatenate(sc_parts)                # [N] noisy instance scores
        cand = np.argpartition(sc, -64)[-64:]
        lex = x[b][cand] @ wei + bei                 # exact f32 recheck
        i = int(cand[int(lex.max(axis=1).argmax())])
        cr = x[b, i] @ we + be                       # exact critical embedding
        crit[b] = cr
        q = cr @ wq + bq
        v = (wk @ q) / scale
        wev = we @ v                                 # [D]
        w8 = _blk_d(wev, WEVS)
        wev8s[2 * b] = w8
        wev8s[2 * b + 1] = w8

    in2 = [
        {"xt": shards[c][0], "xn": shards[c][1], "wev": wev8s[c]}
        for c in range(NCORES)
    ]
    res2 = run_bass_kernel_spmd(_cache["l2"], in2, core_ids=list(range(NCORES))).results

    out = np.zeros((B, C), dtype=np.float32)
    for b in range(B):
        u = res2[2 * b]["u"][0].astype(np.float64) + res2[2 * b + 1]["u"][0]
        S = float(
            res2[2 * b]["ssum"].sum(dtype=np.float64)
            + res2[2 * b + 1]["ssum"].sum(dtype=np.float64)
        )
        wX = u / (XS * S)
        attn_bag = wX @ we + be
        fused = np.concatenate([crit[b], attn_bag])
        out[b] = fused @ wb + bb
    return out


# revision 20
# speedup vs baseline: 1.0684x; 1.0684x over previous
"""DSMIL pooling kernel for 8 Trainium2 NeuronCores — folded-algebra fp8 design.

Algebraic folding (exact): with h = x@we + be,
  inst_logits = h@wi + bi           = x @ (we@wi) + (be@wi + bi)
  scores      = q·(h@wk + bk)/√E    = x @ (we@(wk·q))/√E + const  (const cancels in softmax)
  attn_bag    = Σ attn_n h_n        = (Σ attn_n x_n) @ we + be    (Σ attn_n = 1)
So the device only needs three thin, memory-bound x-contractions:
  L1: l = x @ wei      (per-instance logits; host does argmax + exact f32
      recheck of the top-64 noisy candidates, then critical/q/v folds)
  L2: s = x @ wev  ->  w = exp(s)  ->  wX = Σ w_n x_n   (one launch)
x is streamed in fp8e4m3 (x·16), weights fp8 (wei·512, wev·1024); the host
applies exact power-of-2 unscaling. DoubleRow matmuls contract 256 rows per
instruction (dual-fp8 ldweights need stationary free ≥16 and small dual-dim
strides, hence the padded wei and the n = blk*256 + i*128 + p blocking).
Host glue is O(E²) weight folds plus one O(64·D) recheck.

Sharding: core c <- (bag c//2, half c%2), each shard NS=8192 instances.
"""

import numpy as np
import ml_dtypes

import concourse.mybir as mybir
import concourse.tile as tile
from concourse import bacc
from concourse.bass_utils import run_bass_kernel_spmd

# ---- tile-tail drain workaround (this walrus build rejects >1 sync-wait
# per instruction on the kernel-tail Drain) ----
from concourse.vector_clock import ScopedClock

_MAX_WAITS = 1


def _patched_drain_and_barrier(self, tick_clock, wait_clock):
    probe = self.nc.sync.nop(nofuse=True, hint="tile_drain_waits")
    wait_clock.add_sem_waits(probe.ins, ScopedClock({None: tick_clock.global_clock}))
    si = probe.ins.sync_info
    waits = list(si.on_wait) if si is not None and si.on_wait else []
    if len(waits) > _MAX_WAITS:
        si.on_wait = waits[:_MAX_WAITS]
        rest = waits[_MAX_WAITS:]
        for k in range(0, len(rest), _MAX_WAITS):
            extra = self.nc.sync.nop(nofuse=True, hint="tile_drain_waits")
            esi = extra.ins.sync_info
            if esi is None:
                extra.ins.sync_info = mybir.SyncInfo(
                    on_wait=rest[k : k + _MAX_WAITS], on_update=[]
                )
            else:
                esi.on_wait = rest[k : k + _MAX_WAITS]
    self.nc.sync.drain()
    self.nc.all_engine_barrier()
    popped = self.nc._tile_sem_poison_stack.pop()
    assert popped is self._sem_poison
    self.nc.clear_and_free_semaphores(list(self.sems.allocated().values()))
    self.nc.all_engine_barrier()


tile.TileContext._drain_and_barrier = _patched_drain_and_barrier

F32 = mybir.dt.float32
F8 = mybir.dt.float8e4
FP8 = ml_dtypes.float8_e4m3

B, N, D, E, C = 4, 16384, 1024, 512, 2
NCORES = 8
NS = N // 2          # per-core sequence shard
NT = 512             # n-tile for the xT stream
NTILES = NS // NT    # 16
PR = 4               # 256-row DoubleRow blocks along D (d = pr*256 + i*128 + p)
NBLK = NS // 256     # 32  (n = blk*256 + i*128 + p)
DC = D // 128        # 8
CPAD = 16            # wei columns padded up to the dual-fp8 ldweights minimum

XS = 16.0            # x fp8 scale
WEIS = 512.0         # wei fp8 scale
WEVS = 1024.0        # wev fp8 scale

_cache = {}


def _build_l1():
    nc = bacc.Bacc(None, target_bir_lowering=False)
    xt_d = nc.dram_tensor("xt", [128, NTILES, PR, 2, NT], F8, kind="ExternalInput")
    wei_d = nc.dram_tensor("wei", [128, PR, 2, CPAD], F8, kind="ExternalInput")
    l_d = nc.dram_tensor("l", [C, NTILES, NT], F32, kind="ExternalOutput")

    with tile.TileContext(nc) as tc:
        with (
            tc.tile_pool(name="wp", bufs=1) as wp,
            tc.tile_pool(name="xp", bufs=3) as xp,
            tc.tile_pool(name="ps", bufs=4, space="PSUM") as pp,
        ):
            wei_sb = wp.tile([128, PR, 2, CPAD], F8)
            nc.sync.dma_start(wei_sb[:], wei_d[:])
            lstack = wp.tile([C, NTILES, NT], F32)

            for k in range(NTILES // 2):
                x_t = xp.tile([128, 2, PR, 2, NT], F8, tag="xt", name="xt")
                nc.sync.dma_start(x_t[:], xt_d[:, 2 * k : 2 * k + 2])
                for t2 in range(2):
                    nt = 2 * k + t2
                    ps = pp.tile([CPAD, NT], F32, tag="l")
                    for pr in range(PR):
                        nc.tensor.matmul(
                            ps[:],
                            lhsT=wei_sb[:, pr],
                            rhs=x_t[:, t2, pr],
                            start=(pr == 0),
                            stop=(pr == PR - 1),
                            perf_mode=mybir.MatmulPerfMode.DoubleRow,
                        )
                    nc.scalar.copy(lstack[:, nt, :], ps[0:C, :])
                if k == 5:
                    # ship the first 12 tiles early so only a small output
                    # DMA trails the stream
                    nc.sync.dma_start(l_d[:, 0:12], lstack[:, 0:12])
            nc.sync.dma_start(l_d[:, 12:NTILES], lstack[:, 12:NTILES])
    nc.compile()
    return nc


def _build_l2():
    nc = bacc.Bacc(None, target_bir_lowering=False)
    xt_d = nc.dram_tensor("xt", [128, NTILES, PR, 2, NT], F8, kind="ExternalInput")
    xn_d = nc.dram_tensor("xn", [128, NBLK, 2, D], F8, kind="ExternalInput")
    wev_d = nc.dram_tensor("wev", [128, PR, 2, 1], F8, kind="ExternalInput")
    u_d = nc.dram_tensor("u", [128, DC], F32, kind="ExternalOutput")
    ssum_d = nc.dram_tensor("ssum", [128, NTILES // 2], F32, kind="ExternalOutput")

    NSEG = NTILES // 2        # 8 pipeline stages: xt tile k + xn chunk k
    CB = NBLK // NSEG         # 4 blks per chunk/segment

    with tile.TileContext(nc) as tc:
        with (
            tc.tile_pool(name="wp", bufs=1) as wp,
            tc.tile_pool(name="xp", bufs=3) as xp,
            tc.tile_pool(name="xnp", bufs=3) as xnp,
            tc.tile_pool(name="pw", bufs=1, space="PSUM") as pwp,
            tc.tile_pool(name="pu", bufs=2, space="PSUM") as pup,
        ):
            wev_sb = wp.tile([128, PR, 2, 1], F8)
            nc.sync.dma_start(wev_sb[:], wev_d[:])

            # s = x @ wev lands in n-partition layout via x-as-stationary:
            # wps[p, i, blk] = s[blk*256 + i*128 + p] (raw, scaled XS*WEVS).
            wps = pwp.tile([128, 2, NBLK, 1], F32, tag="w")
            w2d = wp.tile([128, 2, NBLK, 1], F8)
            ssum = wp.tile([128, NSEG], F32)
            u_acc = wp.tile([128, DC], F32)
            nc.vector.memset(u_acc[:], 0.0)

            for k in range(NSEG):
                # paired streams: xt tile k, then xn chunk k (same DMA ring)
                x_t = xp.tile([128, 2, PR, 2, NT], F8, tag="xt", name="xt")
                nc.sync.dma_start(x_t[:], xt_d[:, 2 * k : 2 * k + 2])
                xn_t = xnp.tile([128, CB, 2, D], F8, tag="xn", name="xn")
                nc.sync.dma_start(xn_t[:], xn_d[:, CB * k : CB * (k + 1)])

                # phase A for tile k -> wps blks 4k..4k+3
                for t2 in range(2):
                    nt = 2 * k + t2
                    for jc in range(4):
                        for pr in range(PR):
                            nc.tensor.matmul(
                                wps[:, jc % 2, 2 * nt + jc // 2],
                                lhsT=x_t[:, t2, pr, :, jc * 128 : (jc + 1) * 128],
                                rhs=wev_sb[:, pr],
                                start=(pr == 0),
                                stop=(pr == PR - 1),
                                perf_mode=mybir.MatmulPerfMode.DoubleRow,
                            )
                # elementwise w = exp(s/16384) for this segment (no softmax
                # max barrier: scores are O(0.5))
                nc.scalar.activation(
                    w2d[:, :, CB * k : CB * (k + 1), :],
                    wps[:, :, CB * k : CB * (k + 1), :],
                    mybir.ActivationFunctionType.Exp,
                    scale=1.0 / (XS * WEVS),
                    accum_out=ssum[:, k : k + 1],
                )
                # phase B segment k: u[p, dc] += sum_{blk in chunk} w_n x_n.
                # dc groups run sequentially within one psum bank (interleaved
                # groups within a bank corrupt each other); segments alternate
                # between two banks and drain into u_acc on DVE.
                pb = pup.tile([128, DC, 1], F32, tag="pu", name="pu")
                for dc in range(DC):
                    for j in range(CB):
                        nc.tensor.matmul(
                            pb[:, dc],
                            lhsT=xn_t[:, j, :, dc * 128 : (dc + 1) * 128],
                            rhs=w2d[:, :, CB * k + j],
                            start=(j == 0),
                            stop=(j == CB - 1),
                            perf_mode=mybir.MatmulPerfMode.DoubleRow,
                        )
                nc.vector.tensor_add(u_acc[:], u_acc[:], pb[:, :, 0])

            nc.sync.dma_start(ssum_d[:], ssum[:])
            nc.sync.dma_start(u_d[:], u_acc[:])
    nc.compile()
    return nc


def _q8(a, scale, lim=200.0):
    return np.clip(np.asarray(a, np.float32) * scale, -lim, lim).astype(FP8)


def _prep_x(xs):
    """xs [NS, D] f32 -> (xt8 [128, NTILES, PR, 2, NT], xn8 [128, NBLK, 2, D])."""
    xq = _q8(xs, XS)
    xt8 = np.ascontiguousarray(
        xq.reshape(NTILES, NT, PR, 2, 128).transpose(4, 0, 2, 3, 1)
    )
    xn8 = np.ascontiguousarray(
        xq.reshape(NBLK, 2, 128, D).transpose(2, 0, 1, 3)
    )
    return xt8, xn8


def _blk_d(v, scale):
    """[D, m] f32 -> [128, PR, 2, m] fp8 with d = pr*256 + i*128 + p."""
    v = np.asarray(v, np.float32)
    if v.ndim == 1:
        v = v[:, None]
    m = v.shape[1]
    return np.ascontiguousarray(
        _q8(v, scale).reshape(PR, 2, 128, m).transpose(2, 0, 1, 3)
    )


def kernel(x, we, be, wi, bi, wq, bq, wk, bk, wb, bb):
    x = np.asarray(x, dtype=np.float32)
    we = np.asarray(we, dtype=np.float32)
    be = np.asarray(be, dtype=np.float32)
    wi = np.asarray(wi, dtype=np.float32)
    bi = np.asarray(bi, dtype=np.float32)
    wq = np.asarray(wq, dtype=np.float32)
    bq = np.asarray(bq, dtype=np.float32)
    wk = np.asarray(wk, dtype=np.float32)
    bk = np.asarray(bk, dtype=np.float32)
    wb = np.asarray(wb, dtype=np.float32)
    bb = np.asarray(bb, dtype=np.float32)

    if "l1" not in _cache:
        _cache["l1"] = _build_l1()
    if "l2" not in _cache:
        _cache["l2"] = _build_l2()

    wei = we @ wi                       # [D, C]
    bei = be @ wi + bi                  # [C]
    wei_pad = np.zeros((D, CPAD), np.float32)
    wei_pad[:, :C] = wei
    wei8 = _blk_d(wei_pad, WEIS)

    shards = []                         # per-core (xt8, xn8)
    for c in range(NCORES):
        b, h = divmod(c, 2)
        shards.append(_prep_x(x[b, h * NS : (h + 1) * NS]))

    in1 = [{"xt": s[0], "wei": wei8} for s in shards]
    res1 = run_bass_kernel_spmd(_cache["l1"], in1, core_ids=list(range(NCORES))).results

    # ---- host glue: noisy argmax + exact f32 recheck -> critical -> v ----
    scale = np.float32(E) ** 0.5
    crit = [None] * B
    wev8s = [None] * NCORES
    for b in range(B):
        sc_parts = []
        for h in range(2):
            lraw = res1[2 * b + h]["l"]              # [C, NTILES, NT]
            l = lraw.transpose(1, 2, 0).reshape(NS, C) / (XS * WEIS) + bei
            sc_parts.append(l.max(axis=1))
        sc = np.concatenate(sc_parts)                # [N] noisy instance scores
        cand = np.argpartition(sc, -64)[-64:]
        lex = x[b][cand] @ wei + bei                 # exact f32 recheck
        i = int(cand[int(lex.max(axis=1).argmax())])
        cr = x[b, i] @ we + be                       # exact critical embedding
        crit[b] = cr
        q = cr @ wq + bq
        v = (wk @ q) / scale
        wev = we @ v                                 # [D]
        w8 = _blk_d(wev, WEVS)
        wev8s[2 * b] = w8
        wev8s[2 * b + 1] = w8

    in2 = [
        {"xt": shards[c][0], "xn": shards[c][1], "wev": wev8s[c]}
        for c in range(NCORES)
    ]
    res2 = run_bass_kernel_spmd(_cache["l2"], in2, core_ids=list(range(NCORES))).results

    out = np.zeros((B, C), dtype=np.float32)
    for b in range(B):
        # u[p, dc] = wX[dc*128 + p]
        u = (
            res2[2 * b]["u"].astype(np.float64) + res2[2 * b + 1]["u"]
        ).T.reshape(D)
        S = float(
            res2[2 * b]["ssum"].sum(dtype=np.float64)
            + res2[2 * b + 1]["ssum"].sum(dtype=np.float64)
        )
        wX = u / (XS * S)
        attn_bag = wX @ we + be
        fused = np.concatenate([crit[b], attn_bag])
        out[b] = fused @ wb + bb
    return out
